# revision 26
# baseline (speedup 1.0000x reference)
"""Additive (Bahdanau) attention on 8 TRN2 NeuronCores.

Reference computation (per batch b):
    proj_kv = kv @ W_kv + b_kv                     (S, A)
    proj_q  = q  @ W_q  + b_q                      (Sq, A)
    score[q,s] = sum_a w_v[a] * tanh(proj_kv[s,a] + proj_q[q,a]) + b_v
    weight = softmax(score, axis=s)                (b_v cancels in softmax)
    out[q,:] = sum_s weight[q,s] * kv[s,:]

Sharding: 8 cores, core i handles batch i//4, query rows 128*(i%4) .. +128,
full kv sequence. Purely data parallel; host scatters/gathers.

Per-core dataflow: A=128 on partitions for the tanh stage, so the per-q
projection enters as a per-partition scalar (VectorE tensor_scalar add into
bf16) and one flat ScalarE tanh instruction covers 16 queries — ScalarE is
the roofline engine (~112us of tanh per core). Score dot = col-tiled M=1
matmuls against stationary w_v (4 concurrent via tile_position), relocated
from PSUM partitions {0,32,64,96} to dense (q,s) layout with a
partition-remapping SBUF->SBUF DMA. Everything PE-facing runs in bf16.
"""

import numpy as np

import concourse.bass as bass  # noqa: F401
import concourse.mybir as mybir
import concourse.tile as tile
from concourse import bacc
from concourse.bass_utils import run_bass_kernel_spmd
from concourse.masks import make_identity

B, S, SQ, HID, A = 2, 1024, 512, 256, 128
P = 128
N_CORES = 8
QSH = SQ * B // N_CORES  # 128 query rows per core
F32 = mybir.dt.float32
BF16 = mybir.dt.bfloat16

# tanh chunk sizes (in queries): small first chunks let ScalarE start early,
# small last chunks shrink the exposed post-tanh tail.
CHUNKS = [8, 16, 16, 16, 16, 16, 16, 16, 8]
# Boundary chunks compute tanh(pkv + pq[q]) via the ScalarE bias operand
# (no VectorE adds): ScalarE pays the per-query instruction overhead, but the
# chunk has no DVE dependency, which removes pipeline stalls at the ends.
BIAS_CHUNKS = {0, 8}
NKC = S // P  # 8 kv s-chunks

_nc_cache = None


def _build():
    nc = bacc.Bacc("TRN2", target_bir_lowering=False, debug=False, num_devices=N_CORES)

    kv_d = nc.dram_tensor("attention_kv", [S, HID], F32, kind="ExternalInput")
    q_d = nc.dram_tensor("attention_query", [QSH, HID], F32, kind="ExternalInput")
    wkv_d = nc.dram_tensor("W_kv", [HID, A], F32, kind="ExternalInput")
    bkv_d = nc.dram_tensor("b_kv", [A], F32, kind="ExternalInput")
    wq_d = nc.dram_tensor("W_q", [HID, A], F32, kind="ExternalInput")
    bq_d = nc.dram_tensor("b_q", [A], F32, kind="ExternalInput")
    wv_d = nc.dram_tensor("w_v", [A], F32, kind="ExternalInput")
    outw_d = nc.dram_tensor("out_weight", [QSH, S], F32, kind="ExternalOutput")
    outa_d = nc.dram_tensor("out_attn", [QSH, HID], F32, kind="ExternalOutput")

    TANH = mybir.ActivationFunctionType.Tanh
    EXP = mybir.ActivationFunctionType.Exp

    with tile.TileContext(nc) as tc:
        with (
            tc.tile_pool(name="const", bufs=1) as cp,
            tc.tile_pool(name="work", bufs=3) as wp,
            tc.tile_pool(name="stage", bufs=2) as sp,
            tc.tile_pool(name="kvst", bufs=8) as kp,
            tc.tile_pool(name="psq", bufs=2, space="PSUM") as psq,   # (128,1024)f32 quads: 4 banks
            tc.tile_pool(name="ptp", bufs=4, space="PSUM") as ptp,   # (128,128)bf16 transposes: 4 banks
        ):
            ident = cp.tile([P, P], BF16)
            make_identity(nc, ident[:])

            # ---- small loads & casts -----------------------------------
            # latency-critical query path loads + casts first; wv last
            q_sb = cp.tile([P, HID], F32)
            nc.sync.dma_start(q_sb[:], q_d.ap())
            wq_sb = cp.tile([P, 2, A], F32)
            nc.scalar.dma_start(wq_sb[:], wq_d.ap().rearrange("(c p) a -> p c a", p=P))
            wkv_sb = cp.tile([P, 2, A], F32)
            nc.sync.dma_start(wkv_sb[:], wkv_d.ap().rearrange("(c p) a -> p c a", p=P))
            bkv_sb = cp.tile([P, 1], F32)
            nc.gpsimd.dma_start(bkv_sb[:], bkv_d.ap()[:, None])
            bq_sb = cp.tile([P, 1], F32)
            nc.gpsimd.dma_start(bq_sb[:], bq_d.ap()[:, None])
            wv_sb = cp.tile([P, 1], F32)
            nc.gpsimd.dma_start(wv_sb[:], wv_d.ap()[:, None])
            q_bf = cp.tile([P, HID], BF16)
            nc.vector.tensor_copy(q_bf[:], q_sb[:])
            wq_bf = cp.tile([P, 2, A], BF16)
            nc.vector.tensor_copy(wq_bf[:], wq_sb[:])
            wkv_bf = cp.tile([P, 2, A], BF16)
            nc.vector.tensor_copy(wkv_bf[:], wkv_sb[:])
            bias_sb = cp.tile([P, 1], F32)
            nc.vector.tensor_add(bias_sb[:], bkv_sb[:], bq_sb[:])
            wv_bf = cp.tile([P, 1], BF16)
            nc.vector.tensor_copy(wv_bf[:], wv_sb[:])

            # ---- kv load (2 s-chunks per DMA; ~2us completion latency is
            # per-DMA, so fewer+bigger transfers across both HWDGE rings) ----
            kv_bf = cp.tile([P, NKC, HID], BF16)  # [p, k, h]: kv row k*128+p
            kvT_bf = cp.tile([P, 2, S], BF16)  # [h_in_chunk, c, s]
            kv3 = kv_d.ap().rearrange("(k p) h -> p k h", p=P)
            dma_engines = [nc.sync, nc.scalar]
            pkv_bf = cp.tile([P, S], BF16)  # projT_kv[a, s] (b_kv folded via pq)
            for g in range(2):
                kf = kp.tile([P, 4, HID], F32, tag="kvstage")
                dma_engines[g].dma_start(kf[:], kv3[:, 4 * g : 4 * g + 4, :])
                nc.vector.tensor_copy(kv_bf[:, 4 * g : 4 * g + 4, :], kf[:])
                for kk in range(4):
                    k = 4 * g + kk
                    for c in range(2):
                        tp = ptp.tile([P, P], BF16, tag="tpose")
                        nc.tensor.transpose(
                            tp[:], kv_bf[:, k, c * P : (c + 1) * P], ident[:]
                        )
                        eng = nc.vector.tensor_copy if c == 0 else nc.scalar.copy
                        eng(kvT_bf[:, c, k * P : (k + 1) * P], tp[:])
                    if k % 2 == 1:
                        # projection quarter over the two chunks just transposed
                        n = k // 2
                        tps = psq.tile([P, S], F32, tag="quad")
                        for c in range(2):
                            nc.tensor.matmul(
                                tps[:, :256],
                                wkv_bf[:, c, :],
                                kvT_bf[:, c, n * 256 : (n + 1) * 256],
                                start=(c == 0),
                                stop=(c == 1),
                            )
                        nc.vector.tensor_copy(
                            pkv_bf[:, n * 256 : (n + 1) * 256], tps[:, :256]
                        )

            # ---- projections (A on partitions) -------------------------
            # projT_q[a, q] + b_kv[a] + b_q[a]  (f32 per-partition scalar)
            qT_bf = cp.tile([P, 2, QSH], BF16)
            for c in range(2):
                tp = ptp.tile([P, P], BF16, tag="tpose")
                nc.tensor.transpose(tp[:], q_bf[:, c * P : (c + 1) * P], ident[:])
                nc.vector.tensor_copy(qT_bf[:, c, :], tp[:])
            tpq = psq.tile([P, S], F32, tag="quad")
            for c in range(2):
                nc.tensor.matmul(
                    tpq[:, :QSH],
                    wq_bf[:, c, :],
                    qT_bf[:, c, :],
                    start=(c == 0),
                    stop=(c == 1),
                )
            pq_sb = cp.tile([P, QSH], F32)
            nc.vector.tensor_scalar_add(pq_sb[:], tpq[:, :QSH], bias_sb[:])


            # ---- main loop: tanh cube + score dots ---------------------
            # quad t <-> queries {t, 32+t, 64+t, 96+t}. Chunks cover
            # consecutive quads; j = 4*tl + v in a chunk maps to
            # q = (t0 + tl) + 32*v. Quads are processed in PAIRS sharing one
            # (128, 2048) PSUM tile (pair p0: free [0:1024), p0+1: [1024:2048)).
            score_sb = cp.tile([P, S], F32)  # relocated scores, partition = query
            t0 = 0
            for ci, qc in enumerate(CHUNKS):
                nquad = qc // 4
                hc = wp.tile([P, 16 * S], BF16, tag="hc")
                if ci in BIAS_CHUNKS:
                    for j in range(qc):
                        tl, v = divmod(j, 4)
                        q = (t0 + tl) + 32 * v
                        nc.scalar.activation(
                            hc[:, j * S : (j + 1) * S],
                            pkv_bf[:],
                            TANH,
                            bias=pq_sb[:, q : q + 1],
                        )
                else:
                    for j in range(qc):
                        tl, v = divmod(j, 4)
                        q = (t0 + tl) + 32 * v
                        nc.vector.tensor_scalar_add(
                            hc[:, j * S : (j + 1) * S], pkv_bf[:], pq_sb[:, q : q + 1]
                        )
                    nc.scalar.activation(hc[:, : qc * S], hc[:, : qc * S], TANH)
                last_chunk = t0 + nquad == 32
                for tl in range(nquad):
                    t = t0 + tl
                    quad = psq.tile([P, S], F32, tag="quad")
                    for v in range(4):
                        for n in range(2):
                            nc.tensor.matmul(
                                quad[32 * v : 32 * v + 1, n * 512 : (n + 1) * 512],
                                wv_bf[:],
                                hc[:, (4 * tl + v) * S + n * 512 : (4 * tl + v) * S + (n + 1) * 512],
                                start=True,
                                stop=True,
                                tile_position=(0, 32 * v),
                            )
                    stg = sp.tile([P, S], F32, tag="stage")
                    if last_chunk and tl == nquad - 1:
                        # final quad: stream copy+reloc per half on both DMA
                        # rings so the exp isn't stuck behind one long chain
                        relo_eng = [nc.sync, nc.scalar]
                        for n in range(2):
                            nc.vector.tensor_copy(
                                stg[:, n * 512 : (n + 1) * 512],
                                quad[:, n * 512 : (n + 1) * 512],
                            )
                            relo_eng[n].dma_start(
                                score_sb[t : t + 97 : 32, n * 512 : (n + 1) * 512],
                                stg[0:128:32, n * 512 : (n + 1) * 512],
                            )
                    else:
                        nc.vector.tensor_copy(stg[:], quad[:])
                        nc.sync.dma_start(score_sb[t : t + 97 : 32, :], stg[0:128:32, :])
                t0 += nquad

            # ---- softmax (scores bounded by sum|w_v| <= 11.4: no max sub)
            # exp straight to bf16 for the PE-facing weight transpose; the
            # 1/l normalization is folded into the f32 weight output and the
            # final attention-output copy (both per-partition scalar muls).
            p_bf = cp.tile([P, S], BF16)
            lsum = cp.tile([P, 1], F32)
            nc.scalar.activation(p_bf[:], score_sb[:], EXP, accum_out=lsum[:])
            rec = cp.tile([P, 1], F32)
            nc.vector.reciprocal(rec[:], lsum[:])

            # ---- attention output: out[q,h] = sum_s w[q,s] kv[s,h] -----
            wT_bf = cp.tile([P, NKC, P], BF16)  # [s_in_chunk, k, q] (unnormalized)
            for k in range(NKC):
                tp = ptp.tile([P, P], BF16, tag="tpose")
                nc.tensor.transpose(tp[:], p_bf[:, k * P : (k + 1) * P], ident[:])
                nc.vector.tensor_copy(wT_bf[:, k, :], tp[:])
            out_ps = psq.tile([P, S], F32, tag="quad")
            for k in range(NKC):
                nc.tensor.matmul(
                    out_ps[:, :HID],
                    wT_bf[:, k, :],
                    kv_bf[:, k, :],
                    start=(k == 0),
                    stop=(k == NKC - 1),
                )
            out_sb = cp.tile([P, HID], F32)
            nc.vector.tensor_scalar_mul(out_sb[:], out_ps[:, :HID], rec[:])
            nc.sync.dma_start(outa_d.ap(), out_sb[:])

            # weight output (f32), off the critical path; halves on both rings
            w_sb = cp.tile([P, S], F32)
            w_eng = [nc.scalar, nc.sync]
            for n in range(2):
                nc.vector.tensor_scalar_mul(
                    w_sb[:, n * 512 : (n + 1) * 512],
                    p_bf[:, n * 512 : (n + 1) * 512],
                    rec[:],
                )
                w_eng[n].dma_start(
                    outw_d.ap()[:, n * 512 : (n + 1) * 512],
                    w_sb[:, n * 512 : (n + 1) * 512],
                )

    nc.compile()
    return nc


def _get_nc():
    global _nc_cache
    if _nc_cache is None:
        _nc_cache = _build()
    return _nc_cache


def _make_in_maps(attention_kv, attention_query, W_kv, b_kv, W_q, b_q, w_v):
    kv = np.ascontiguousarray(np.asarray(attention_kv, dtype=np.float32))
    qq = np.ascontiguousarray(np.asarray(attention_query, dtype=np.float32))
    wkv = np.ascontiguousarray(np.asarray(W_kv, dtype=np.float32))
    bkv = np.ascontiguousarray(np.asarray(b_kv, dtype=np.float32))
    wq = np.ascontiguousarray(np.asarray(W_q, dtype=np.float32))
    bq = np.ascontiguousarray(np.asarray(b_q, dtype=np.float32))
    wv = np.ascontiguousarray(np.asarray(w_v, dtype=np.float32))
    in_maps = []
    for i in range(N_CORES):
        b, qi = divmod(i, N_CORES // B)
        in_maps.append(
            {
                "attention_kv": kv[b],
                "attention_query": np.ascontiguousarray(
                    qq[b, qi * QSH : (qi + 1) * QSH]
                ),
                "W_kv": wkv,
                "b_kv": bkv,
                "W_q": wq,
                "b_q": bq,
                "w_v": wv,
            }
        )
    return in_maps


def _gather(results):
    out = np.empty((B, SQ, HID), np.float32)
    wgt = np.empty((B, SQ, S), np.float32)
    for i in range(N_CORES):
        b, qi = divmod(i, N_CORES // B)
        out[b, qi * QSH : (qi + 1) * QSH] = results[i]["out_attn"]
        wgt[b, qi * QSH : (qi + 1) * QSH] = results[i]["out_weight"]
    return out, wgt


def run(in_maps, **kwargs):
    """Compile (cached) + execute on 8 cores. Returns BassKernelResults."""
    return run_bass_kernel_spmd(
        _get_nc(), in_maps, core_ids=list(range(N_CORES)), **kwargs
    )


def kernel(
    attention_kv, attention_query, W_kv, b_kv, W_q, b_q, w_v, b_v=None, **_unused
):
    # b_v shifts every score by the same constant, so it cancels in the
    # softmax and never affects either returned tensor.
    in_maps = _make_in_maps(attention_kv, attention_query, W_kv, b_kv, W_q, b_q, w_v)
    res = run(in_maps)
    return _gather(res.results)


# revision 27
# speedup vs baseline: 1.0066x; 1.0066x over previous
"""Additive (Bahdanau) attention on 8 TRN2 NeuronCores.

Reference computation (per batch b):
    proj_kv = kv @ W_kv + b_kv                     (S, A)
    proj_q  = q  @ W_q  + b_q                      (Sq, A)
    score[q,s] = sum_a w_v[a] * tanh(proj_kv[s,a] + proj_q[q,a]) + b_v
    weight = softmax(score, axis=s)                (b_v cancels in softmax)
    out[q,:] = sum_s weight[q,s] * kv[s,:]

Sharding: 8 cores, core i handles batch i//4 and query rows 128*(i%4)..+128
against the full kv sequence — purely data parallel, host scatters/gathers,
no collectives.

Per-core dataflow (ScalarE is the roofline engine: 16.8M tanh evaluations =
~110us at 128 lanes / 1.2 GHz; everything else hides under that stream):

  head:  kv loaded in two 512KB DMAs (Sync+Scalar HWDGE rings), cast to
         bf16, transposed 128x128-wise on PE so A=128 lands on partitions;
         projection matmul quarters are interleaved into the transpose
         stream. Query path identical but tiny. Biases fold into projT_q.
  loop:  chunks of 16 queries: VectorE broadcast-adds projT_q[q] (bf16
         tensor_scalar, per-partition scalar) onto projT_kv, one flat
         ScalarE tanh instruction per chunk, then per query two col-tiled
         M=1 matmuls against stationary w_v (tile_position=(0,32v), 4
         concurrent) write score rows to PSUM partitions {0,32,64,96};
         each quad of rows is copied to SBUF and relocated to partitions
         {t,32+t,64+t,96+t} by a partition-remapping SBUF->SBUF DMA.
         Boundary chunks (first/last 8 queries) instead use the ScalarE
         bias operand (no VectorE dependency) to avoid ramp stalls.
  tail:  one exp over (128,1024) with fused accum row-sum, reciprocal,
         PE transpose of the bf16 probabilities, 8 accumulated matmuls
         against kv (bf16), 1/l folded into the output copies; weight
         output normalized to f32 and DMAd in halves on both rings.

Numerics: PE-facing data is bf16 (scores accumulate in f32 PSUM; softmax
and outputs in f32) -> rel err ~2.5e-3 on both outputs. exp() needs no
max-subtraction: |score| <= sum|w_v| <= 128/sqrt(128) = 11.4.
"""

import numpy as np

import concourse.bass as bass  # noqa: F401
import concourse.mybir as mybir
import concourse.tile as tile
from concourse import bacc
from concourse.bass_utils import run_bass_kernel_spmd
from concourse.masks import make_identity

B, S, SQ, HID, A = 2, 1024, 512, 256, 128
P = 128
N_CORES = 8
QSH = SQ * B // N_CORES  # 128 query rows per core
F32 = mybir.dt.float32
BF16 = mybir.dt.bfloat16

# tanh chunk sizes (in queries): small first chunks let ScalarE start early,
# small last chunks shrink the exposed post-tanh tail.
CHUNKS = [8, 16, 16, 16, 16, 16, 16, 16, 8]
# Boundary chunks compute tanh(pkv + pq[q]) via the ScalarE bias operand
# (no VectorE adds): ScalarE pays the per-query instruction overhead, but the
# chunk has no DVE dependency, which removes pipeline stalls at the ends.
BIAS_CHUNKS = {0, 8}
NKC = S // P  # 8 kv s-chunks

_nc_cache = None


def _build():
    nc = bacc.Bacc("TRN2", target_bir_lowering=False, debug=False, num_devices=N_CORES)

    kv_d = nc.dram_tensor("attention_kv", [S, HID], F32, kind="ExternalInput")
    q_d = nc.dram_tensor("attention_query", [QSH, HID], F32, kind="ExternalInput")
    wkv_d = nc.dram_tensor("W_kv", [HID, A], F32, kind="ExternalInput")
    bkv_d = nc.dram_tensor("b_kv", [A], F32, kind="ExternalInput")
    wq_d = nc.dram_tensor("W_q", [HID, A], F32, kind="ExternalInput")
    bq_d = nc.dram_tensor("b_q", [A], F32, kind="ExternalInput")
    wv_d = nc.dram_tensor("w_v", [A], F32, kind="ExternalInput")
    outw_d = nc.dram_tensor("out_weight", [QSH, S], F32, kind="ExternalOutput")
    outa_d = nc.dram_tensor("out_attn", [QSH, HID], F32, kind="ExternalOutput")

    TANH = mybir.ActivationFunctionType.Tanh
    EXP = mybir.ActivationFunctionType.Exp

    with tile.TileContext(nc) as tc:
        with (
            tc.tile_pool(name="const", bufs=1) as cp,
            tc.tile_pool(name="work", bufs=2) as wp,
            tc.tile_pool(name="stage", bufs=2) as sp,
            tc.tile_pool(name="kvst", bufs=8) as kp,
            tc.tile_pool(name="psq", bufs=2, space="PSUM") as psq,   # (128,1024)f32 quads: 4 banks
            tc.tile_pool(name="ptp", bufs=4, space="PSUM") as ptp,   # (128,128)bf16 transposes: 4 banks
        ):
            ident = cp.tile([P, P], BF16)
            make_identity(nc, ident[:])

            # ---- small loads & casts -----------------------------------
            # latency-critical query path loads + casts first; wv last
            q_sb = cp.tile([P, HID], F32)
            nc.sync.dma_start(q_sb[:], q_d.ap())
            wq_sb = cp.tile([P, 2, A], F32)
            nc.scalar.dma_start(wq_sb[:], wq_d.ap().rearrange("(c p) a -> p c a", p=P))
            wkv_sb = cp.tile([P, 2, A], F32)
            nc.sync.dma_start(wkv_sb[:], wkv_d.ap().rearrange("(c p) a -> p c a", p=P))
            bkv_sb = cp.tile([P, 1], F32)
            nc.gpsimd.dma_start(bkv_sb[:], bkv_d.ap()[:, None])
            bq_sb = cp.tile([P, 1], F32)
            nc.gpsimd.dma_start(bq_sb[:], bq_d.ap()[:, None])
            wv_sb = cp.tile([P, 1], F32)
            nc.gpsimd.dma_start(wv_sb[:], wv_d.ap()[:, None])
            q_bf = cp.tile([P, HID], BF16)
            nc.vector.tensor_copy(q_bf[:], q_sb[:])
            wq_bf = cp.tile([P, 2, A], BF16)
            nc.vector.tensor_copy(wq_bf[:], wq_sb[:])
            wkv_bf = cp.tile([P, 2, A], BF16)
            nc.vector.tensor_copy(wkv_bf[:], wkv_sb[:])
            bias_sb = cp.tile([P, 1], F32)
            nc.vector.tensor_add(bias_sb[:], bkv_sb[:], bq_sb[:])
            wv_bf = cp.tile([P, 1], BF16)
            nc.vector.tensor_copy(wv_bf[:], wv_sb[:])

            # ---- kv load (2 s-chunks per DMA; ~2us completion latency is
            # per-DMA, so fewer+bigger transfers across both HWDGE rings) ----
            kv_bf = cp.tile([P, NKC, HID], BF16)  # [p, k, h]: kv row k*128+p
            kvT_bf = cp.tile([P, 2, S], BF16)  # [h_in_chunk, c, s]
            kv3 = kv_d.ap().rearrange("(k p) h -> p k h", p=P)
            dma_engines = [nc.sync, nc.scalar]
            pkv_bf = cp.tile([P, S], BF16)  # projT_kv[a, s] (b_kv folded via pq)
            for g in range(2):
                kf = kp.tile([P, 4, HID], F32, tag="kvstage")
                dma_engines[g].dma_start(kf[:], kv3[:, 4 * g : 4 * g + 4, :])
                nc.vector.tensor_copy(kv_bf[:, 4 * g : 4 * g + 4, :], kf[:])
                for kk in range(4):
                    k = 4 * g + kk
                    for c in range(2):
                        tp = ptp.tile([P, P], BF16, tag="tpose")
                        nc.tensor.transpose(
                            tp[:], kv_bf[:, k, c * P : (c + 1) * P], ident[:]
                        )
                        eng = nc.vector.tensor_copy if c == 0 else nc.scalar.copy
                        eng(kvT_bf[:, c, k * P : (k + 1) * P], tp[:])
                    if k % 2 == 1:
                        # projection quarter over the two chunks just transposed
                        n = k // 2
                        tps = psq.tile([P, S], F32, tag="quad")
                        for c in range(2):
                            nc.tensor.matmul(
                                tps[:, :256],
                                wkv_bf[:, c, :],
                                kvT_bf[:, c, n * 256 : (n + 1) * 256],
                                start=(c == 0),
                                stop=(c == 1),
                            )
                        nc.vector.tensor_copy(
                            pkv_bf[:, n * 256 : (n + 1) * 256], tps[:, :256]
                        )

            # ---- projections (A on partitions) -------------------------
            # projT_q[a, q] + b_kv[a] + b_q[a]  (f32 per-partition scalar)
            qT_bf = cp.tile([P, 2, QSH], BF16)
            for c in range(2):
                tp = ptp.tile([P, P], BF16, tag="tpose")
                nc.tensor.transpose(tp[:], q_bf[:, c * P : (c + 1) * P], ident[:])
                nc.vector.tensor_copy(qT_bf[:, c, :], tp[:])
            tpq = psq.tile([P, S], F32, tag="quad")
            for c in range(2):
                nc.tensor.matmul(
                    tpq[:, :QSH],
                    wq_bf[:, c, :],
                    qT_bf[:, c, :],
                    start=(c == 0),
                    stop=(c == 1),
                )
            pq_sb = cp.tile([P, QSH], F32)
            nc.vector.tensor_scalar_add(pq_sb[:], tpq[:, :QSH], bias_sb[:])


            # ---- main loop: tanh cube + score dots ---------------------
            # quad t <-> queries {t, 32+t, 64+t, 96+t}. Chunks cover
            # consecutive quads; j = 4*tl + v in a chunk maps to
            # q = (t0 + tl) + 32*v. Quads are processed in PAIRS sharing one
            # (128, 2048) PSUM tile (pair p0: free [0:1024), p0+1: [1024:2048)).
            score_sb = cp.tile([P, S], F32)  # relocated scores, partition = query
            t0 = 0
            for ci, qc in enumerate(CHUNKS):
                nquad = qc // 4
                hc = wp.tile([P, 16 * S], BF16, tag="hc")
                if ci in BIAS_CHUNKS:
                    for j in range(qc):
                        tl, v = divmod(j, 4)
                        q = (t0 + tl) + 32 * v
                        nc.scalar.activation(
                            hc[:, j * S : (j + 1) * S],
                            pkv_bf[:],
                            TANH,
                            bias=pq_sb[:, q : q + 1],
                        )
                else:
                    for j in range(qc):
                        tl, v = divmod(j, 4)
                        q = (t0 + tl) + 32 * v
                        nc.vector.tensor_scalar_add(
                            hc[:, j * S : (j + 1) * S], pkv_bf[:], pq_sb[:, q : q + 1]
                        )
                    nc.scalar.activation(hc[:, : qc * S], hc[:, : qc * S], TANH)
                last_chunk = t0 + nquad == 32
                for tl in range(nquad):
                    t = t0 + tl
                    quad = psq.tile([P, S], F32, tag="quad")
                    for v in range(4):
                        for n in range(2):
                            nc.tensor.matmul(
                                quad[32 * v : 32 * v + 1, n * 512 : (n + 1) * 512],
                                wv_bf[:],
                                hc[:, (4 * tl + v) * S + n * 512 : (4 * tl + v) * S + (n + 1) * 512],
                                start=True,
                                stop=True,
                                tile_position=(0, 32 * v),
                            )
                    stg = sp.tile([P, S], F32, tag="stage")
                    if last_chunk and tl == nquad - 1:
                        # final quad: stream copy+reloc per half on both DMA
                        # rings so the exp isn't stuck behind one long chain
                        relo_eng = [nc.sync, nc.scalar]
                        for n in range(2):
                            nc.vector.tensor_copy(
                                stg[:, n * 512 : (n + 1) * 512],
                                quad[:, n * 512 : (n + 1) * 512],
                            )
                            relo_eng[n].dma_start(
                                score_sb[t : t + 97 : 32, n * 512 : (n + 1) * 512],
                                stg[0:128:32, n * 512 : (n + 1) * 512],
                            )
                    else:
                        nc.vector.tensor_copy(stg[:], quad[:])
                        nc.sync.dma_start(score_sb[t : t + 97 : 32, :], stg[0:128:32, :])
                t0 += nquad

            # ---- softmax (scores bounded by sum|w_v| <= 11.4: no max sub)
            # exp straight to bf16 for the PE-facing weight transpose; the
            # 1/l normalization is folded into the f32 weight output and the
            # final attention-output copy (both per-partition scalar muls).
            p_bf = cp.tile([P, S], BF16)
            lsum = cp.tile([P, 1], F32)
            nc.scalar.activation(p_bf[:], score_sb[:], EXP, accum_out=lsum[:])
            rec = cp.tile([P, 1], F32)
            nc.vector.reciprocal(rec[:], lsum[:])

            # ---- attention output: out[q,h] = sum_s w[q,s] kv[s,h] -----
            wT_bf = cp.tile([P, NKC, P], BF16)  # [s_in_chunk, k, q] (unnormalized)
            for k in range(NKC):
                tp = ptp.tile([P, P], BF16, tag="tpose")
                nc.tensor.transpose(tp[:], p_bf[:, k * P : (k + 1) * P], ident[:])
                nc.vector.tensor_copy(wT_bf[:, k, :], tp[:])
            out_ps = psq.tile([P, S], F32, tag="quad")
            for k in range(NKC):
                nc.tensor.matmul(
                    out_ps[:, :HID],
                    wT_bf[:, k, :],
                    kv_bf[:, k, :],
                    start=(k == 0),
                    stop=(k == NKC - 1),
                )
            out_sb = cp.tile([P, HID], F32)
            nc.vector.tensor_scalar_mul(out_sb[:], out_ps[:, :HID], rec[:])
            nc.sync.dma_start(outa_d.ap(), out_sb[:])

            # weight output (f32), off the critical path; halves on both rings
            w_sb = cp.tile([P, S], F32)
            w_eng = [nc.scalar, nc.sync]
            for n in range(2):
                nc.vector.tensor_scalar_mul(
                    w_sb[:, n * 512 : (n + 1) * 512],
                    p_bf[:, n * 512 : (n + 1) * 512],
                    rec[:],
                )
                w_eng[n].dma_start(
                    outw_d.ap()[:, n * 512 : (n + 1) * 512],
                    w_sb[:, n * 512 : (n + 1) * 512],
                )

    nc.compile()
    return nc


def _get_nc():
    global _nc_cache
    if _nc_cache is None:
        _nc_cache = _build()
    return _nc_cache


def _make_in_maps(attention_kv, attention_query, W_kv, b_kv, W_q, b_q, w_v):
    kv = np.ascontiguousarray(np.asarray(attention_kv, dtype=np.float32))
    qq = np.ascontiguousarray(np.asarray(attention_query, dtype=np.float32))
    wkv = np.ascontiguousarray(np.asarray(W_kv, dtype=np.float32))
    bkv = np.ascontiguousarray(np.asarray(b_kv, dtype=np.float32))
    wq = np.ascontiguousarray(np.asarray(W_q, dtype=np.float32))
    bq = np.ascontiguousarray(np.asarray(b_q, dtype=np.float32))
    wv = np.ascontiguousarray(np.asarray(w_v, dtype=np.float32))
    in_maps = []
    for i in range(N_CORES):
        b, qi = divmod(i, N_CORES // B)
        in_maps.append(
            {
                "attention_kv": kv[b],
                "attention_query": np.ascontiguousarray(
                    qq[b, qi * QSH : (qi + 1) * QSH]
                ),
                "W_kv": wkv,
                "b_kv": bkv,
                "W_q": wq,
                "b_q": bq,
                "w_v": wv,
            }
        )
    return in_maps


def _gather(results):
    out = np.empty((B, SQ, HID), np.float32)
    wgt = np.empty((B, SQ, S), np.float32)
    for i in range(N_CORES):
        b, qi = divmod(i, N_CORES // B)
        out[b, qi * QSH : (qi + 1) * QSH] = results[i]["out_attn"]
        wgt[b, qi * QSH : (qi + 1) * QSH] = results[i]["out_weight"]
    return out, wgt


def run(in_maps, **kwargs):
    """Compile (cached) + execute on 8 cores. Returns BassKernelResults."""
    return run_bass_kernel_spmd(
        _get_nc(), in_maps, core_ids=list(range(N_CORES)), **kwargs
    )


def kernel(
    attention_kv, attention_query, W_kv, b_kv, W_q, b_q, w_v, b_v=None, **_unused
):
    # b_v shifts every score by the same constant, so it cancels in the
    # softmax and never affects either returned tensor.
    in_maps = _make_in_maps(attention_kv, attention_query, W_kv, b_kv, W_q, b_q, w_v)
    res = run(in_maps)
    return _gather(res.results)


# revision 28
# speedup vs baseline: 1.0087x; 1.0021x over previous
"""Additive (Bahdanau) attention on 8 TRN2 NeuronCores.

Reference computation (per batch b):
    proj_kv = kv @ W_kv + b_kv                     (S, A)
    proj_q  = q  @ W_q  + b_q                      (Sq, A)
    score[q,s] = sum_a w_v[a] * tanh(proj_kv[s,a] + proj_q[q,a]) + b_v
    weight = softmax(score, axis=s)                (b_v cancels in softmax)
    out[q,:] = sum_s weight[q,s] * kv[s,:]

Sharding: 8 cores, core i handles batch i//4 and query rows 128*(i%4)..+128
against the full kv sequence — purely data parallel, host scatters/gathers,
no collectives.

Per-core dataflow (ScalarE is the roofline engine: 16.8M tanh evaluations =
~110us at 128 lanes / 1.2 GHz; everything else hides under that stream):

  head:  kv loaded in two 512KB DMAs (Sync+Scalar HWDGE rings), cast to
         bf16, transposed 128x128-wise on PE so A=128 lands on partitions;
         projection matmul quarters are interleaved into the transpose
         stream. Query path identical but tiny. Biases fold into projT_q.
  loop:  chunks of 16 queries: VectorE broadcast-adds projT_q[q] (bf16
         tensor_scalar, per-partition scalar) onto projT_kv, one flat
         ScalarE tanh instruction per chunk, then per query two col-tiled
         M=1 matmuls against stationary w_v (tile_position=(0,32v), 4
         concurrent) write score rows to PSUM partitions {0,32,64,96};
         each quad of rows is copied to SBUF and relocated to partitions
         {t,32+t,64+t,96+t} by a partition-remapping SBUF->SBUF DMA.
         Boundary chunks (first/last 8 queries) instead use the ScalarE
         bias operand (no VectorE dependency) to avoid ramp stalls.
  tail:  one exp over (128,1024) with fused accum row-sum, reciprocal,
         PE transpose of the bf16 probabilities, 8 accumulated matmuls
         against kv (bf16), 1/l folded into the output copies; weight
         output normalized to f32 and DMAd in halves on both rings.

Numerics: PE-facing data is bf16 (scores accumulate in f32 PSUM; softmax
and outputs in f32) -> rel err ~2.5e-3 on both outputs. exp() needs no
max-subtraction: |score| <= sum|w_v| <= 128/sqrt(128) = 11.4.
"""

import numpy as np

import concourse.bass as bass  # noqa: F401
import concourse.mybir as mybir
import concourse.tile as tile
from concourse import bacc
from concourse.bass_utils import run_bass_kernel_spmd
from concourse.masks import make_identity

B, S, SQ, HID, A = 2, 1024, 512, 256, 128
P = 128
N_CORES = 8
QSH = SQ * B // N_CORES  # 128 query rows per core
F32 = mybir.dt.float32
BF16 = mybir.dt.bfloat16

# tanh chunk sizes (in queries): small first chunks let ScalarE start early,
# small last chunks shrink the exposed post-tanh tail.
CHUNKS = [8, 16, 16, 16, 16, 16, 16, 16, 8]
# Boundary chunks compute tanh(pkv + pq[q]) via the ScalarE bias operand
# (no VectorE adds): ScalarE pays the per-query instruction overhead, but the
# chunk has no DVE dependency, which removes pipeline stalls at the ends.
BIAS_CHUNKS = {0, 8}
NKC = S // P  # 8 kv s-chunks

_nc_cache = None


def _build():
    nc = bacc.Bacc("TRN2", target_bir_lowering=False, debug=False, num_devices=N_CORES)

    kv_d = nc.dram_tensor("attention_kv", [S, HID], F32, kind="ExternalInput")
    q_d = nc.dram_tensor("attention_query", [QSH, HID], F32, kind="ExternalInput")
    wkv_d = nc.dram_tensor("W_kv", [HID, A], F32, kind="ExternalInput")
    bkv_d = nc.dram_tensor("b_kv", [A], F32, kind="ExternalInput")
    wq_d = nc.dram_tensor("W_q", [HID, A], F32, kind="ExternalInput")
    bq_d = nc.dram_tensor("b_q", [A], F32, kind="ExternalInput")
    wv_d = nc.dram_tensor("w_v", [A], F32, kind="ExternalInput")
    outw_d = nc.dram_tensor("out_weight", [QSH, S], F32, kind="ExternalOutput")
    outa_d = nc.dram_tensor("out_attn", [QSH, HID], F32, kind="ExternalOutput")

    TANH = mybir.ActivationFunctionType.Tanh
    EXP = mybir.ActivationFunctionType.Exp

    with tile.TileContext(nc) as tc:
        with (
            tc.tile_pool(name="const", bufs=1) as cp,
            tc.tile_pool(name="work", bufs=2) as wp,
            tc.tile_pool(name="stage", bufs=2) as sp,
            tc.tile_pool(name="kvst", bufs=8) as kp,
            tc.tile_pool(name="psq", bufs=2, space="PSUM") as psq,   # (128,1024)f32 quads: 4 banks
            tc.tile_pool(name="ptp", bufs=4, space="PSUM") as ptp,   # (128,128)bf16 transposes: 4 banks
        ):
            ident = cp.tile([P, P], BF16)
            make_identity(nc, ident[:])

            # ---- small loads & casts -----------------------------------
            # latency-critical query path loads + casts first; wv last
            q_sb = cp.tile([P, HID], F32)
            nc.sync.dma_start(q_sb[:], q_d.ap())
            wq_sb = cp.tile([P, 2, A], F32)
            nc.scalar.dma_start(wq_sb[:], wq_d.ap().rearrange("(c p) a -> p c a", p=P))
            wkv_sb = cp.tile([P, 2, A], F32)
            nc.sync.dma_start(wkv_sb[:], wkv_d.ap().rearrange("(c p) a -> p c a", p=P))
            bkv_sb = cp.tile([P, 1], F32)
            nc.gpsimd.dma_start(bkv_sb[:], bkv_d.ap()[:, None])
            bq_sb = cp.tile([P, 1], F32)
            nc.gpsimd.dma_start(bq_sb[:], bq_d.ap()[:, None])
            wv_sb = cp.tile([P, 1], F32)
            nc.gpsimd.dma_start(wv_sb[:], wv_d.ap()[:, None])
            q_bf = cp.tile([P, HID], BF16)
            nc.vector.tensor_copy(q_bf[:], q_sb[:])
            wq_bf = cp.tile([P, 2, A], BF16)
            nc.vector.tensor_copy(wq_bf[:], wq_sb[:])
            wkv_bf = cp.tile([P, 2, A], BF16)
            nc.vector.tensor_copy(wkv_bf[:], wkv_sb[:])
            bias_sb = cp.tile([P, 1], F32)
            nc.vector.tensor_add(bias_sb[:], bkv_sb[:], bq_sb[:])
            wv_bf = cp.tile([P, 1], BF16)
            nc.vector.tensor_copy(wv_bf[:], wv_sb[:])

            # ---- kv load (2 s-chunks per DMA; ~2us completion latency is
            # per-DMA, so fewer+bigger transfers across both HWDGE rings) ----
            kv_bf = cp.tile([P, NKC, HID], BF16)  # [p, k, h]: kv row k*128+p
            kvT_bf = cp.tile([P, 2, S], BF16)  # [h_in_chunk, c, s]
            kv3 = kv_d.ap().rearrange("(k p) h -> p k h", p=P)
            dma_engines = [nc.sync, nc.scalar]
            pkv_bf = cp.tile([P, S], BF16)  # projT_kv[a, s] (b_kv folded via pq)
            for g in range(2):
                kf = kp.tile([P, 4, HID], F32, tag="kvstage")
                dma_engines[g].dma_start(kf[:], kv3[:, 4 * g : 4 * g + 4, :])
                nc.vector.tensor_copy(kv_bf[:, 4 * g : 4 * g + 4, :], kf[:])
                for kk in range(4):
                    k = 4 * g + kk
                    for c in range(2):
                        tp = ptp.tile([P, P], BF16, tag="tpose")
                        nc.tensor.transpose(
                            tp[:], kv_bf[:, k, c * P : (c + 1) * P], ident[:]
                        )
                        eng = nc.vector.tensor_copy if c == 0 else nc.scalar.copy
                        eng(kvT_bf[:, c, k * P : (k + 1) * P], tp[:])
                    if k % 2 == 1:
                        # projection quarter over the two chunks just transposed
                        n = k // 2
                        tps = psq.tile([P, S], F32, tag="quad")
                        for c in range(2):
                            nc.tensor.matmul(
                                tps[:, :256],
                                wkv_bf[:, c, :],
                                kvT_bf[:, c, n * 256 : (n + 1) * 256],
                                start=(c == 0),
                                stop=(c == 1),
                            )
                        nc.vector.tensor_copy(
                            pkv_bf[:, n * 256 : (n + 1) * 256], tps[:, :256]
                        )

            # ---- projections (A on partitions) -------------------------
            # projT_q[a, q] + b_kv[a] + b_q[a]  (f32 per-partition scalar)
            qT_bf = cp.tile([P, 2, QSH], BF16)
            for c in range(2):
                tp = ptp.tile([P, P], BF16, tag="tpose")
                nc.tensor.transpose(tp[:], q_bf[:, c * P : (c + 1) * P], ident[:])
                nc.vector.tensor_copy(qT_bf[:, c, :], tp[:])
            tpq = psq.tile([P, S], F32, tag="quad")
            for c in range(2):
                nc.tensor.matmul(
                    tpq[:, :QSH],
                    wq_bf[:, c, :],
                    qT_bf[:, c, :],
                    start=(c == 0),
                    stop=(c == 1),
                )
            pq_sb = cp.tile([P, QSH], F32)
            nc.vector.tensor_scalar_add(pq_sb[:], tpq[:, :QSH], bias_sb[:])


            # ---- main loop: tanh cube + score dots ---------------------
            # quad t <-> queries {t, 32+t, 64+t, 96+t}. Chunks cover
            # consecutive quads; j = 4*tl + v in a chunk maps to
            # q = (t0 + tl) + 32*v.
            score_sb = cp.tile([P, S], F32)  # relocated scores, partition = query
            t0 = 0
            for ci, qc in enumerate(CHUNKS):
                nquad = qc // 4
                hc = wp.tile([P, 16 * S], BF16, tag="hc")
                if ci in BIAS_CHUNKS:
                    for j in range(qc):
                        tl, v = divmod(j, 4)
                        q = (t0 + tl) + 32 * v
                        nc.scalar.activation(
                            hc[:, j * S : (j + 1) * S],
                            pkv_bf[:],
                            TANH,
                            bias=pq_sb[:, q : q + 1],
                        )
                else:
                    for j in range(qc):
                        tl, v = divmod(j, 4)
                        q = (t0 + tl) + 32 * v
                        nc.vector.tensor_scalar_add(
                            hc[:, j * S : (j + 1) * S], pkv_bf[:], pq_sb[:, q : q + 1]
                        )
                    nc.scalar.activation(hc[:, : qc * S], hc[:, : qc * S], TANH)
                last_chunk = t0 + nquad == 32
                for tl in range(nquad):
                    t = t0 + tl
                    quad = psq.tile([P, S], F32, tag="quad")
                    for v in range(4):
                        for n in range(2):
                            nc.tensor.matmul(
                                quad[32 * v : 32 * v + 1, n * 512 : (n + 1) * 512],
                                wv_bf[:],
                                hc[:, (4 * tl + v) * S + n * 512 : (4 * tl + v) * S + (n + 1) * 512],
                                start=True,
                                stop=True,
                                tile_position=(0, 32 * v),
                            )
                    stg = sp.tile([P, S], F32, tag="stage")
                    if last_chunk and tl == nquad - 1:
                        # final quad: stream copy+reloc per half on both DMA
                        # rings so the exp isn't stuck behind one long chain
                        relo_eng = [nc.sync, nc.scalar]
                        for n in range(2):
                            nc.vector.tensor_copy(
                                stg[:, n * 512 : (n + 1) * 512],
                                quad[:, n * 512 : (n + 1) * 512],
                            )
                            relo_eng[n].dma_start(
                                score_sb[t : t + 97 : 32, n * 512 : (n + 1) * 512],
                                stg[0:128:32, n * 512 : (n + 1) * 512],
                            )
                    else:
                        nc.vector.tensor_copy(stg[:], quad[:])
                        nc.sync.dma_start(score_sb[t : t + 97 : 32, :], stg[0:128:32, :])
                t0 += nquad

            # ---- softmax (scores bounded by sum|w_v| <= 11.4: no max sub)
            # exp straight to bf16 for the PE-facing weight transpose; the
            # 1/l normalization is folded into the f32 weight output and the
            # final attention-output copy (both per-partition scalar muls).
            p_bf = cp.tile([P, S], BF16)
            lsum = cp.tile([P, 1], F32)
            nc.scalar.activation(p_bf[:], score_sb[:], EXP, accum_out=lsum[:])
            rec = cp.tile([P, 1], F32)
            nc.vector.reciprocal(rec[:], lsum[:])

            # ---- attention output: out[q,h] = sum_s w[q,s] kv[s,h] -----
            wT_bf = cp.tile([P, NKC, P], BF16)  # [s_in_chunk, k, q] (unnormalized)
            for k in range(NKC):
                tp = ptp.tile([P, P], BF16, tag="tpose")
                nc.tensor.transpose(tp[:], p_bf[:, k * P : (k + 1) * P], ident[:])
                nc.vector.tensor_copy(wT_bf[:, k, :], tp[:])
            out_ps = psq.tile([P, S], F32, tag="quad")
            for k in range(NKC):
                nc.tensor.matmul(
                    out_ps[:, :HID],
                    wT_bf[:, k, :],
                    kv_bf[:, k, :],
                    start=(k == 0),
                    stop=(k == NKC - 1),
                )
            out_sb = cp.tile([P, HID], F32)
            nc.vector.tensor_scalar_mul(out_sb[:], out_ps[:, :HID], rec[:])
            nc.sync.dma_start(outa_d.ap(), out_sb[:])

            # weight output (f32), off the critical path; halves on both rings
            w_sb = cp.tile([P, S], F32)
            w_eng = [nc.scalar, nc.sync]
            for n in range(2):
                nc.vector.tensor_scalar_mul(
                    w_sb[:, n * 512 : (n + 1) * 512],
                    p_bf[:, n * 512 : (n + 1) * 512],
                    rec[:],
                )
                w_eng[n].dma_start(
                    outw_d.ap()[:, n * 512 : (n + 1) * 512],
                    w_sb[:, n * 512 : (n + 1) * 512],
                )

    nc.compile()
    return nc


def _get_nc():
    global _nc_cache
    if _nc_cache is None:
        _nc_cache = _build()
    return _nc_cache


def _make_in_maps(attention_kv, attention_query, W_kv, b_kv, W_q, b_q, w_v):
    kv = np.ascontiguousarray(np.asarray(attention_kv, dtype=np.float32))
    qq = np.ascontiguousarray(np.asarray(attention_query, dtype=np.float32))
    wkv = np.ascontiguousarray(np.asarray(W_kv, dtype=np.float32))
    bkv = np.ascontiguousarray(np.asarray(b_kv, dtype=np.float32))
    wq = np.ascontiguousarray(np.asarray(W_q, dtype=np.float32))
    bq = np.ascontiguousarray(np.asarray(b_q, dtype=np.float32))
    wv = np.ascontiguousarray(np.asarray(w_v, dtype=np.float32))
    in_maps = []
    for i in range(N_CORES):
        b, qi = divmod(i, N_CORES // B)
        in_maps.append(
            {
                "attention_kv": kv[b],
                "attention_query": np.ascontiguousarray(
                    qq[b, qi * QSH : (qi + 1) * QSH]
                ),
                "W_kv": wkv,
                "b_kv": bkv,
                "W_q": wq,
                "b_q": bq,
                "w_v": wv,
            }
        )
    return in_maps


def _gather(results):
    out = np.empty((B, SQ, HID), np.float32)
    wgt = np.empty((B, SQ, S), np.float32)
    for i in range(N_CORES):
        b, qi = divmod(i, N_CORES // B)
        out[b, qi * QSH : (qi + 1) * QSH] = results[i]["out_attn"]
        wgt[b, qi * QSH : (qi + 1) * QSH] = results[i]["out_weight"]
    return out, wgt


def run(in_maps, **kwargs):
    """Compile (cached) + execute on 8 cores. Returns BassKernelResults."""
    return run_bass_kernel_spmd(
        _get_nc(), in_maps, core_ids=list(range(N_CORES)), **kwargs
    )


def kernel(
    attention_kv, attention_query, W_kv, b_kv, W_q, b_q, w_v, b_v=None, **_unused
):
    # b_v shifts every score by the same constant, so it cancels in the
    # softmax and never affects either returned tensor.
    in_maps = _make_in_maps(attention_kv, attention_query, W_kv, b_kv, W_q, b_q, w_v)
    res = run(in_maps)
    return _gather(res.results)


# revision 29
# speedup vs baseline: 1.0118x; 1.0030x over previous
"""Additive (Bahdanau) attention on 8 TRN2 NeuronCores.

Reference computation (per batch b):
    proj_kv = kv @ W_kv + b_kv                     (S, A)
    proj_q  = q  @ W_q  + b_q                      (Sq, A)
    score[q,s] = sum_a w_v[a] * tanh(proj_kv[s,a] + proj_q[q,a]) + b_v
    weight = softmax(score, axis=s)                (b_v cancels in softmax)
    out[q,:] = sum_s weight[q,s] * kv[s,:]

Sharding: 8 cores, core i handles batch i//4 and query rows 128*(i%4)..+128
against the full kv sequence — purely data parallel, host scatters/gathers,
no collectives.

Per-core dataflow (ScalarE is the roofline engine: 16.8M tanh evaluations =
~110us at 128 lanes / 1.2 GHz; everything else hides under that stream):

  head:  kv loaded in two 512KB DMAs (Sync+Scalar HWDGE rings), cast to
         bf16, transposed 128x128-wise on PE so A=128 lands on partitions;
         projection matmul quarters are interleaved into the transpose
         stream. Query path identical but tiny. Biases fold into projT_q.
  loop:  chunks of 16 queries: VectorE broadcast-adds projT_q[q] (bf16
         tensor_scalar, per-partition scalar) onto projT_kv, one flat
         ScalarE tanh instruction per chunk, then per query two col-tiled
         M=1 matmuls against stationary w_v (tile_position=(0,32v), 4
         concurrent) write score rows to PSUM partitions {0,32,64,96};
         each quad of rows is copied to SBUF and relocated to partitions
         {t,32+t,64+t,96+t} by a partition-remapping SBUF->SBUF DMA.
         Boundary chunks (first/last 8 queries) instead use the ScalarE
         bias operand (no VectorE dependency) to avoid ramp stalls.
  tail:  one exp over (128,1024) with fused accum row-sum, reciprocal,
         PE transpose of the bf16 probabilities, 8 accumulated matmuls
         against kv (bf16), 1/l folded into the output copies; weight
         output normalized to f32 and DMAd in halves on both rings.

Numerics: PE-facing data is bf16 (scores accumulate in f32 PSUM; softmax
and outputs in f32) -> rel err ~2.5e-3 on both outputs. exp() needs no
max-subtraction: |score| <= sum|w_v| <= 128/sqrt(128) = 11.4.
"""

import numpy as np

import concourse.bass as bass  # noqa: F401
import concourse.mybir as mybir
import concourse.tile as tile
from concourse import bacc
from concourse.bass_utils import run_bass_kernel_spmd
from concourse.masks import make_identity

B, S, SQ, HID, A = 2, 1024, 512, 256, 128
P = 128
N_CORES = 8
QSH = SQ * B // N_CORES  # 128 query rows per core
F32 = mybir.dt.float32
BF16 = mybir.dt.bfloat16

# tanh chunk sizes (in queries): small first chunks let ScalarE start early,
# small last chunks shrink the exposed post-tanh tail.
CHUNKS = [8, 16, 16, 16, 16, 16, 16, 16, 8]
# Boundary chunks compute tanh(pkv + pq[q]) via the ScalarE bias operand
# (no VectorE adds): ScalarE pays the per-query instruction overhead, but the
# chunk has no DVE dependency, which removes pipeline stalls at the ends.
BIAS_CHUNKS = {0, 8}
NKC = S // P  # 8 kv s-chunks

_nc_cache = None


def _build():
    nc = bacc.Bacc("TRN2", target_bir_lowering=False, debug=False, num_devices=N_CORES)

    kv_d = nc.dram_tensor("attention_kv", [S, HID], F32, kind="ExternalInput")
    q_d = nc.dram_tensor("attention_query", [QSH, HID], F32, kind="ExternalInput")
    wkv_d = nc.dram_tensor("W_kv", [HID, A], F32, kind="ExternalInput")
    bkv_d = nc.dram_tensor("b_kv", [A], F32, kind="ExternalInput")
    wq_d = nc.dram_tensor("W_q", [HID, A], F32, kind="ExternalInput")
    bq_d = nc.dram_tensor("b_q", [A], F32, kind="ExternalInput")
    wv_d = nc.dram_tensor("w_v", [A], F32, kind="ExternalInput")
    outw_d = nc.dram_tensor("out_weight", [QSH, S], F32, kind="ExternalOutput")
    outa_d = nc.dram_tensor("out_attn", [QSH, HID], F32, kind="ExternalOutput")

    TANH = mybir.ActivationFunctionType.Tanh
    EXP = mybir.ActivationFunctionType.Exp

    with tile.TileContext(nc) as tc:
        with (
            tc.tile_pool(name="const", bufs=1) as cp,
            tc.tile_pool(name="work", bufs=2) as wp,
            tc.tile_pool(name="stage", bufs=3) as sp,
            tc.tile_pool(name="kvst", bufs=8) as kp,
            tc.tile_pool(name="psq", bufs=2, space="PSUM") as psq,   # (128,1024)f32 quads: 4 banks
            tc.tile_pool(name="ptp", bufs=4, space="PSUM") as ptp,   # (128,128)bf16 transposes: 4 banks
        ):
            ident = cp.tile([P, P], BF16)
            make_identity(nc, ident[:])

            # ---- small loads & casts -----------------------------------
            # latency-critical query path loads + casts first; wv last
            q_sb = cp.tile([P, HID], F32)
            nc.sync.dma_start(q_sb[:], q_d.ap())
            wq_sb = cp.tile([P, 2, A], F32)
            nc.scalar.dma_start(wq_sb[:], wq_d.ap().rearrange("(c p) a -> p c a", p=P))
            wkv_sb = cp.tile([P, 2, A], F32)
            nc.sync.dma_start(wkv_sb[:], wkv_d.ap().rearrange("(c p) a -> p c a", p=P))
            bkv_sb = cp.tile([P, 1], F32)
            nc.gpsimd.dma_start(bkv_sb[:], bkv_d.ap()[:, None])
            bq_sb = cp.tile([P, 1], F32)
            nc.gpsimd.dma_start(bq_sb[:], bq_d.ap()[:, None])
            wv_sb = cp.tile([P, 1], F32)
            nc.gpsimd.dma_start(wv_sb[:], wv_d.ap()[:, None])
            q_bf = cp.tile([P, HID], BF16)
            nc.vector.tensor_copy(q_bf[:], q_sb[:])
            wq_bf = cp.tile([P, 2, A], BF16)
            nc.vector.tensor_copy(wq_bf[:], wq_sb[:])
            wkv_bf = cp.tile([P, 2, A], BF16)
            nc.vector.tensor_copy(wkv_bf[:], wkv_sb[:])
            bias_sb = cp.tile([P, 1], F32)
            nc.vector.tensor_add(bias_sb[:], bkv_sb[:], bq_sb[:])
            wv_bf = cp.tile([P, 1], BF16)
            nc.vector.tensor_copy(wv_bf[:], wv_sb[:])

            # ---- kv load (2 s-chunks per DMA; ~2us completion latency is
            # per-DMA, so fewer+bigger transfers across both HWDGE rings) ----
            kv_bf = cp.tile([P, NKC, HID], BF16)  # [p, k, h]: kv row k*128+p
            kvT_bf = cp.tile([P, 2, S], BF16)  # [h_in_chunk, c, s]
            kv3 = kv_d.ap().rearrange("(k p) h -> p k h", p=P)
            dma_engines = [nc.sync, nc.scalar]
            pkv_bf = cp.tile([P, S], BF16)  # projT_kv[a, s] (b_kv folded via pq)
            for g in range(2):
                kf = kp.tile([P, 4, HID], F32, tag="kvstage")
                dma_engines[g].dma_start(kf[:], kv3[:, 4 * g : 4 * g + 4, :])
                nc.vector.tensor_copy(kv_bf[:, 4 * g : 4 * g + 4, :], kf[:])
                for kk in range(4):
                    k = 4 * g + kk
                    for c in range(2):
                        tp = ptp.tile([P, P], BF16, tag="tpose")
                        nc.tensor.transpose(
                            tp[:], kv_bf[:, k, c * P : (c + 1) * P], ident[:]
                        )
                        eng = nc.vector.tensor_copy if c == 0 else nc.scalar.copy
                        eng(kvT_bf[:, c, k * P : (k + 1) * P], tp[:])
                    if k % 2 == 1:
                        # projection quarter over the two chunks just transposed
                        n = k // 2
                        tps = psq.tile([P, S], F32, tag="quad")
                        for c in range(2):
                            nc.tensor.matmul(
                                tps[:, :256],
                                wkv_bf[:, c, :],
                                kvT_bf[:, c, n * 256 : (n + 1) * 256],
                                start=(c == 0),
                                stop=(c == 1),
                            )
                        nc.vector.tensor_copy(
                            pkv_bf[:, n * 256 : (n + 1) * 256], tps[:, :256]
                        )

            # ---- projections (A on partitions) -------------------------
            # projT_q[a, q] + b_kv[a] + b_q[a]  (f32 per-partition scalar)
            qT_bf = cp.tile([P, 2, QSH], BF16)
            for c in range(2):
                tp = ptp.tile([P, P], BF16, tag="tpose")
                nc.tensor.transpose(tp[:], q_bf[:, c * P : (c + 1) * P], ident[:])
                nc.vector.tensor_copy(qT_bf[:, c, :], tp[:])
            tpq = psq.tile([P, S], F32, tag="quad")
            for c in range(2):
                nc.tensor.matmul(
                    tpq[:, :QSH],
                    wq_bf[:, c, :],
                    qT_bf[:, c, :],
                    start=(c == 0),
                    stop=(c == 1),
                )
            pq_sb = cp.tile([P, QSH], F32)
            nc.vector.tensor_scalar_add(pq_sb[:], tpq[:, :QSH], bias_sb[:])


            # ---- main loop: tanh cube + score dots ---------------------
            # quad t <-> queries {t, 32+t, 64+t, 96+t}. Chunks cover
            # consecutive quads; j = 4*tl + v in a chunk maps to
            # q = (t0 + tl) + 32*v.
            score_sb = cp.tile([P, S], F32)  # relocated scores, partition = query
            t0 = 0
            for ci, qc in enumerate(CHUNKS):
                nquad = qc // 4
                hc = wp.tile([P, 16 * S], BF16, tag="hc")
                if ci in BIAS_CHUNKS:
                    for j in range(qc):
                        tl, v = divmod(j, 4)
                        q = (t0 + tl) + 32 * v
                        nc.scalar.activation(
                            hc[:, j * S : (j + 1) * S],
                            pkv_bf[:],
                            TANH,
                            bias=pq_sb[:, q : q + 1],
                        )
                else:
                    for j in range(qc):
                        tl, v = divmod(j, 4)
                        q = (t0 + tl) + 32 * v
                        nc.vector.tensor_scalar_add(
                            hc[:, j * S : (j + 1) * S], pkv_bf[:], pq_sb[:, q : q + 1]
                        )
                    nc.scalar.activation(hc[:, : qc * S], hc[:, : qc * S], TANH)
                last_chunk = t0 + nquad == 32
                for tl in range(nquad):
                    t = t0 + tl
                    quad = psq.tile([P, S], F32, tag="quad")
                    for v in range(4):
                        for n in range(2):
                            nc.tensor.matmul(
                                quad[32 * v : 32 * v + 1, n * 512 : (n + 1) * 512],
                                wv_bf[:],
                                hc[:, (4 * tl + v) * S + n * 512 : (4 * tl + v) * S + (n + 1) * 512],
                                start=True,
                                stop=True,
                                tile_position=(0, 32 * v),
                            )
                    stg = sp.tile([P, S], F32, tag="stage")
                    if last_chunk and tl == nquad - 1:
                        # final quad: stream copy+reloc per half on both DMA
                        # rings so the exp isn't stuck behind one long chain
                        relo_eng = [nc.sync, nc.scalar]
                        for n in range(2):
                            nc.vector.tensor_copy(
                                stg[:, n * 512 : (n + 1) * 512],
                                quad[:, n * 512 : (n + 1) * 512],
                            )
                            relo_eng[n].dma_start(
                                score_sb[t : t + 97 : 32, n * 512 : (n + 1) * 512],
                                stg[0:128:32, n * 512 : (n + 1) * 512],
                            )
                    else:
                        nc.vector.tensor_copy(stg[:], quad[:])
                        nc.sync.dma_start(score_sb[t : t + 97 : 32, :], stg[0:128:32, :])
                t0 += nquad

            # ---- softmax (scores bounded by sum|w_v| <= 11.4: no max sub)
            # exp straight to bf16 for the PE-facing weight transpose; the
            # 1/l normalization is folded into the f32 weight output and the
            # final attention-output copy (both per-partition scalar muls).
            p_bf = cp.tile([P, S], BF16)
            lsum = cp.tile([P, 1], F32)
            nc.scalar.activation(p_bf[:], score_sb[:], EXP, accum_out=lsum[:])
            rec = cp.tile([P, 1], F32)
            nc.vector.reciprocal(rec[:], lsum[:])

            # ---- attention output: out[q,h] = sum_s w[q,s] kv[s,h] -----
            wT_bf = cp.tile([P, NKC, P], BF16)  # [s_in_chunk, k, q] (unnormalized)
            for k in range(NKC):
                tp = ptp.tile([P, P], BF16, tag="tpose")
                nc.tensor.transpose(tp[:], p_bf[:, k * P : (k + 1) * P], ident[:])
                eng = nc.vector.tensor_copy if k % 2 == 0 else nc.scalar.copy
                eng(wT_bf[:, k, :], tp[:])
            out_ps = psq.tile([P, S], F32, tag="quad")
            for k in range(NKC):
                nc.tensor.matmul(
                    out_ps[:, :HID],
                    wT_bf[:, k, :],
                    kv_bf[:, k, :],
                    start=(k == 0),
                    stop=(k == NKC - 1),
                )
            out_sb = cp.tile([P, HID], F32)
            nc.vector.tensor_scalar_mul(out_sb[:], out_ps[:, :HID], rec[:])
            nc.sync.dma_start(outa_d.ap(), out_sb[:])

            # weight output (f32), off the critical path; halves on both rings
            w_sb = cp.tile([P, S], F32)
            w_eng = [nc.scalar, nc.sync]
            for n in range(2):
                nc.vector.tensor_scalar_mul(
                    w_sb[:, n * 512 : (n + 1) * 512],
                    p_bf[:, n * 512 : (n + 1) * 512],
                    rec[:],
                )
                w_eng[n].dma_start(
                    outw_d.ap()[:, n * 512 : (n + 1) * 512],
                    w_sb[:, n * 512 : (n + 1) * 512],
                )

    nc.compile()
    return nc


def _get_nc():
    global _nc_cache
    if _nc_cache is None:
        _nc_cache = _build()
    return _nc_cache


def _make_in_maps(attention_kv, attention_query, W_kv, b_kv, W_q, b_q, w_v):
    kv = np.ascontiguousarray(np.asarray(attention_kv, dtype=np.float32))
    qq = np.ascontiguousarray(np.asarray(attention_query, dtype=np.float32))
    wkv = np.ascontiguousarray(np.asarray(W_kv, dtype=np.float32))
    bkv = np.ascontiguousarray(np.asarray(b_kv, dtype=np.float32))
    wq = np.ascontiguousarray(np.asarray(W_q, dtype=np.float32))
    bq = np.ascontiguousarray(np.asarray(b_q, dtype=np.float32))
    wv = np.ascontiguousarray(np.asarray(w_v, dtype=np.float32))
    in_maps = []
    for i in range(N_CORES):
        b, qi = divmod(i, N_CORES // B)
        in_maps.append(
            {
                "attention_kv": kv[b],
                "attention_query": np.ascontiguousarray(
                    qq[b, qi * QSH : (qi + 1) * QSH]
                ),
                "W_kv": wkv,
                "b_kv": bkv,
                "W_q": wq,
                "b_q": bq,
                "w_v": wv,
            }
        )
    return in_maps


def _gather(results):
    out = np.empty((B, SQ, HID), np.float32)
    wgt = np.empty((B, SQ, S), np.float32)
    for i in range(N_CORES):
        b, qi = divmod(i, N_CORES // B)
        out[b, qi * QSH : (qi + 1) * QSH] = results[i]["out_attn"]
        wgt[b, qi * QSH : (qi + 1) * QSH] = results[i]["out_weight"]
    return out, wgt


def run(in_maps, **kwargs):
    """Compile (cached) + execute on 8 cores. Returns BassKernelResults."""
    return run_bass_kernel_spmd(
        _get_nc(), in_maps, core_ids=list(range(N_CORES)), **kwargs
    )


def kernel(
    attention_kv, attention_query, W_kv, b_kv, W_q, b_q, w_v, b_v=None, **_unused
):
    # b_v shifts every score by the same constant, so it cancels in the
    # softmax and never affects either returned tensor.
    in_maps = _make_in_maps(attention_kv, attention_query, W_kv, b_kv, W_q, b_q, w_v)
    res = run(in_maps)
    return _gather(res.results)
